# revision 4
# baseline (speedup 1.0000x reference)
"""Per-class mean (segment reduce) on 8 Trainium2 NeuronCores.

Algorithm
---------
out[c] = sum_{i: labels[i]==c} features[i] / max(count_c, 1),  C=1000, A=512.

Rows are split evenly across the 8 cores.  On the host each core's rows
are bucketed by class *window* w = c >> 7 (8 windows of 128 classes =
1024 >= 1000 -> the 8 PSUM banks) and laid out window-major, padded so
every window starts on a 128-row tile boundary.  Features are cast to
bf16 (2 B/elem; the one-hot weights below are exact and PSUM accumulates
in fp32, so the only inexactness is the bf16 cast, ~2^-9 relative per
element -> ~1e-3 on the final class means).  The per-core tensor is
stored partition-major [128, T, 512]: row t*128+p lives at [p, t, :], so
the device streams it with plain contiguous DMA - no gather.

Each 128-row tile is window-pure.  A tiny [128, T] bf16 slot table
(slot = label & 127, -1 for padding) rides along; the DVE builds each
tile's one-hot [128 rows x 128 slots] on-chip with a single
tensor_scalar(is_equal) against an iota, and the PE does one bf16
matmul per tile:

    psum_bank[w] += onehot_t.T @ feat_tile          # fp32 PSUM

Windows are contiguous in the tile stream, so each PSUM bank closes in
order and is copied + DMA'd out overlapping the next window's matmuls.
The host adds the 8 per-core partials and divides by the global counts
(np.bincount), matching the reference order (sum, then divide).

One SPMD program serves all 8 cores: the schedule depends only on the
cross-core max tile count per window; per-core data (features, slot
table) are inputs.  Compiled at call time, memoized per schedule.
"""

import functools
import sys
import types

import numpy as np

N_CORES = 8
NUM_CLASSES = 1000
N_WINDOWS = 8          # class windows of 128 -> 8 PSUM banks
A_DIM = 512
K_TILES = 8            # 128-row tiles per DMA chunk (8 KB/partition)
N_BUFS = 6             # chunk double-buffering depth
OH_BUFS = 4            # one-hot chunk buffers


def _install_axon_hooks_shim():
    """The slim agent image lacks antenv.axon_hooks; concourse imports it
    when tracing.  Provide a fallback so imports never fail."""
    if "antenv.axon_hooks" in sys.modules:
        return
    try:
        from trn_agent_boot.trn_boot import _ntff_profile_via_ctypes
        hook = _ntff_profile_via_ctypes("/opt/axon/libaxon_pjrt.so")
    except Exception:
        hook = None
    mod = types.ModuleType("antenv.axon_hooks")
    mod.get_axon_ntff_profile_hook = lambda: hook
    mod.set_axon_ntff_profile_hook = lambda h: None
    sys.modules["antenv.axon_hooks"] = mod
    # tracing tries to upload artifacts to shared storage; keep it local
    try:
        import concourse.bass_utils as _bu
        _bu.upload_artifacts = lambda tmpdir: tmpdir
    except Exception:
        pass


@functools.lru_cache(maxsize=4)
def _build_program(w_tiles: tuple):
    """Trace + compile the SPMD Bass program for one schedule."""
    _install_axon_hooks_shim()
    import concourse.bacc as bacc
    import concourse.tile as tile
    from concourse import mybir

    F32 = mybir.dt.float32
    BF16 = mybir.dt.bfloat16
    T = sum(w_tiles)

    # window of each tile + first/last tile per window
    win_of, first_t, last_t = [], {}, {}
    for w in range(N_WINDOWS):
        for _ in range(w_tiles[w]):
            ti = len(win_of)
            win_of.append(w)
            first_t.setdefault(w, ti)
            last_t[w] = ti

    nc = bacc.Bacc("TRN2", target_bir_lowering=False, debug=False)
    feat = nc.declare_dram_parameter("feat", [128, T * A_DIM], BF16,
                                     isOutput=False)
    slots = nc.declare_dram_parameter("slots", [128, T], F32,
                                      isOutput=False)
    out_sums = nc.declare_dram_parameter("out_sums", [N_WINDOWS * 128, A_DIM],
                                         F32, isOutput=True)
    featv = feat[:].rearrange("p (t e) -> p t e", e=A_DIM)

    with tile.TileContext(nc) as tc:
        with (
            tc.tile_pool(name="cst", bufs=1) as cst,
            tc.tile_pool(name="gb", bufs=N_BUFS) as gb_pool,
            tc.tile_pool(name="ohp", bufs=OH_BUFS) as oh_pool,
            tc.tile_pool(name="ps", bufs=1, space="PSUM") as ps_pool,
            tc.tile_pool(name="stg", bufs=1) as stg_pool,
        ):
            slots_sb = cst.tile([128, T], F32, tag="slots_sb")
            nc.sync.dma_start(slots_sb[:], slots[:])
            iota_b = cst.tile([128, 128], BF16, tag="iota_b")
            nc.gpsimd.iota(iota_b[:], pattern=[[1, 128]], base=0,
                           channel_multiplier=0,
                           allow_small_or_imprecise_dtypes=True)

            psum = [ps_pool.tile([128, A_DIM], F32, tag=f"ps_{w}",
                                 name=f"ps_{w}")
                    for w in range(N_WINDOWS)]
            staging = stg_pool.tile([128, N_WINDOWS, A_DIM], F32, tag="stg")

            for c0 in range(0, T, K_TILES):
                cc = min(K_TILES, T - c0)
                gt = gb_pool.tile([128, K_TILES, A_DIM], BF16, tag="gt")
                nc.sync.dma_start(gt[:, :cc, :], featv[:, c0:c0 + cc, :])
                oh = oh_pool.tile([128, K_TILES, 128], BF16, tag="oh")
                for k in range(cc):
                    ti = c0 + k
                    w = win_of[ti]
                    nc.vector.tensor_scalar(
                        oh[:, k, :], iota_b[:], slots_sb[:, ti:ti + 1], None,
                        op0=mybir.AluOpType.is_equal)
                    nc.tensor.matmul(psum[w][:], oh[:, k, :], gt[:, k, :],
                                     start=(ti == first_t[w]),
                                     stop=(ti == last_t[w]))
                    if ti == last_t[w]:
                        # window w final: copy out of PSUM and stream to
                        # DRAM now, overlapping remaining work
                        nc.scalar.copy(staging[:, w, :], psum[w][:])
                        nc.gpsimd.dma_start(
                            out_sums[w * 128:(w + 1) * 128, :],
                            staging[:, w, :])

    nc.compile()
    return nc


def _schedule(labels_all: np.ndarray):
    """Cross-core tile counts per window from labels only."""
    n = labels_all.shape[0]
    n_loc = n // N_CORES
    win = (labels_all.astype(np.int64) >> 7).reshape(N_CORES, n_loc)
    counts = np.stack([np.bincount(win[c], minlength=N_WINDOWS)
                       for c in range(N_CORES)])          # [cores, windows]
    w_tiles = tuple(int(-(-counts[:, w].max() // 128))
                    for w in range(N_WINDOWS))
    return n_loc, w_tiles, win, counts


def make_inputs(features: np.ndarray, labels_np: np.ndarray):
    """Full host prep: schedule + per-core input tensors."""
    import ml_dtypes
    bf16 = ml_dtypes.bfloat16

    n_loc, w_tiles, win, _ = _schedule(labels_np)
    T = sum(w_tiles)
    off_el = np.concatenate([[0], np.cumsum(w_tiles)])[:N_WINDOWS] * 128

    lab_all = labels_all = labels_np.astype(np.int64).reshape(N_CORES, n_loc)
    in_maps = []
    for c in range(N_CORES):
        lab = lab_all[c]
        wc = win[c]
        slot = lab & 127
        order = np.argsort(wc, kind="stable")
        sw = wc[order]
        cnt = np.bincount(wc, minlength=N_WINDOWS)
        cum = np.concatenate([[0], np.cumsum(cnt)])
        rank = np.arange(n_loc) - cum[sw]
        s = off_el[sw] + rank
        p, t = s % 128, s // 128

        f32 = np.ascontiguousarray(
            features[c * n_loc:(c + 1) * n_loc]).astype(np.float32, copy=False)
        feat_host = np.zeros((128, T, A_DIM), dtype=bf16)
        feat_host[p, t] = f32[order].astype(bf16)
        slots_host = np.full((128, T), -1.0, dtype=np.float32)
        slots_host[p, t] = slot[order].astype(np.float32)
        in_maps.append({"feat": feat_host.reshape(128, T * A_DIM),
                        "slots": slots_host})
    return n_loc, w_tiles, in_maps


last_run = None    # BassKernelResults of the most recent kernel() call
_last_state = None  # (nc, in_maps) of the most recent kernel() call


def rerun(n=1, trace=True):
    """Re-execute the last-compiled program on the same inputs; returns
    the list of exec_time_ns (requires a prior kernel() call)."""
    from concourse.bass_utils import run_bass_kernel_spmd
    nc, in_maps = _last_state
    times = []
    for _ in range(n):
        r = run_bass_kernel_spmd(nc, in_maps, list(range(N_CORES)),
                                 trace=trace)
        times.append(r.exec_time_ns)
    return times


def kernel(features: np.ndarray, labels: np.ndarray) -> np.ndarray:
    global last_run, _last_state
    _install_axon_hooks_shim()
    from concourse.bass_utils import run_bass_kernel_spmd

    features = np.asarray(features)
    labels_np = np.asarray(labels)
    n, a = features.shape
    assert a == A_DIM and n % N_CORES == 0

    n_loc, w_tiles, in_maps = make_inputs(features, labels_np)
    nc = _build_program(w_tiles)

    res = run_bass_kernel_spmd(nc, in_maps, list(range(N_CORES)))
    last_run = res
    _last_state = (nc, in_maps)
    total = np.zeros((N_WINDOWS * 128, A_DIM), dtype=np.float32)
    for c in range(N_CORES):
        total += res.results[c]["out_sums"]
    for w in range(N_WINDOWS):      # windows with no rows anywhere: force 0
        if w_tiles[w] == 0:
            total[w * 128:(w + 1) * 128] = 0.0

    counts = np.bincount(labels_np.astype(np.int64), minlength=NUM_CLASSES)
    counts = np.maximum(counts[:NUM_CLASSES], 1).astype(np.float32)
    return total[:NUM_CLASSES] / counts[:, None]
